# revision 1
# baseline (speedup 1.0000x reference)
"""Chamfer rate-distortion loss on 8 TRN2 NeuronCores.

Layout: 8 cores = 4 batches x 2 chamfer directions. Each core computes, for
its (batch, direction), the per-point nearest-neighbor squared distance of
8192 query points X against 8192 reference points Y.

Device algorithm per core:
  - X and Y are pre-sorted (host) along coordinate AXIS.
  - matmul trick (fp16 hi/lo split, K=11, full PE rate, ~1e-5 abs precision):
    PSUM[m,p] = SCALE^2*(|y_p|^2 - 2 x_m.y_p) = SCALE^2*(D[m,p] - |x_m|^2).
    DVE reduce_min along the free axis gives the per-query band minimum
    (|x|^2 added and rescaled on host).
  - 256 col-tiled sub-chunks of 32 sorted queries (4 per 128-partition PSUM
    block via tile_position) each scan a BAND-wide window of sorted Y around
    their own sorted position; edges padded with duplicates of the extreme
    real points (which can never lower a min below the true min).
  - 16 overflow windows scan the FULL Y for up to 128 "hard" points whose
    nearest neighbor may fall outside their band (selected on host with a
    conservative Morton-neighbor upper bound).

Exactness: for a query x, every Y outside its band differs from x along the
sort axis by at least gap(x), so any excluded point has D >= gap^2.  Host
verifies band_min_D <= gap^2 per point (sound, data-independent); the rare
unverified points are recomputed exactly on host (on expected data: none).
"""

import os

import numpy as np

B, M, P = 4, 8192, 8192
AXIS = 2
SUB = 32             # sub-chunk: 32 sorted queries share one band
BAND = 416           # uniform band width (rt cols) per sub-chunk
PAD = 192            # edge-dup pad = guaranteed halfwidth; band = [32u, 32u+416)
NBLK = 64            # blocks of 128 queries = 4 col-tiled sub-chunks
GRP = 4              # blocks per PSUM tile / per reduce op
OBAND = 512          # overflow window width
NOVER = 16           # overflow windows of 512: full 8192 scan
NOUT = NBLK + NOVER
WT_W = M + 128       # sorted queries + 128 overflow slots
RT_W = P + 2 * PAD   # pad + sorted refs + pad
KROWS = 11           # fp16 hi/lo decomposition rows (see _prep_core)
SCALE = 32.0         # coordinate pre-scale; device min is SCALE^2 * real
LMBDA = 5.0

_CACHE = {}
LAST_RESULTS = None


def _build_bass():
    import concourse.tile as tile
    from concourse import bacc, mybir

    nc = bacc.Bacc(None, target_bir_lowering=False, debug=False)
    f32 = mybir.dt.float32
    f16 = mybir.dt.float16

    wr_d = nc.dram_tensor("wr", [KROWS, WT_W + RT_W], f16, kind="ExternalInput")
    out_d = nc.dram_tensor("out", [128, NOUT], f32, kind="ExternalOutput")

    with tile.TileContext(nc) as tc:
        with (
            tc.tile_pool(name="const", bufs=1) as cpool,
            tc.tile_pool(name="outp", bufs=1) as opool,
            tc.tile_pool(name="psum", bufs=2, space="PSUM") as ppool,
        ):
            wr = cpool.tile([KROWS, WT_W + RT_W], f16)
            # head pieces (first ~48 sub-chunks' weights + bands) land first
            # so the PE can start while the bulk of the input streams in
            HW_, HR_ = 1536, 2048
            nc.sync.dma_start(wr[:, 0:HW_], wr_d[:, 0:HW_])
            nc.sync.dma_start(wr[:, WT_W:WT_W + HR_], wr_d[:, WT_W:WT_W + HR_])
            nc.sync.dma_start(wr[:, HW_:WT_W], wr_d[:, HW_:WT_W])
            nc.sync.dma_start(wr[:, WT_W + HR_:], wr_d[:, WT_W + HR_:])
            outt = opool.tile([128, NOUT], f32)

            for k in range(NBLK // GRP):
                # group stride 512 keeps every matmul output bank-aligned;
                # only cols [0, BAND) are written and reduced
                ps = ppool.tile([128, GRP, 512], f32, tag="ps")
                for g in range(GRP):
                    for s in range(4):
                        u = 4 * (GRP * k + g) + s   # global sub-chunk index
                        nc.tensor.matmul(
                            ps[32 * s:32 * s + 32, g, 0:BAND],
                            wr[:, SUB * u:SUB * u + SUB],
                            wr[:, WT_W + SUB * u:WT_W + SUB * u + BAND],
                            start=True, stop=True,
                            tile_position=(0, 32 * s),
                        )
                nc.vector.tensor_reduce(outt[:, GRP * k:GRP * (k + 1)],
                                        ps[:, :, 0:BAND],
                                        axis=mybir.AxisListType.X,
                                        op=mybir.AluOpType.min)

            for k in range(NOVER // GRP):
                ps = ppool.tile([128, GRP, OBAND], f32, tag="ps")
                for g in range(GRP):
                    j = GRP * k + g   # overflow window index
                    rcol = PAD + OBAND * j
                    nc.tensor.matmul(ps[:, g, :], wr[:, M:M + 128],
                                     wr[:, WT_W + rcol:WT_W + rcol + OBAND],
                                     start=True, stop=True)
                nc.vector.tensor_reduce(
                    outt[:, NBLK + GRP * k:NBLK + GRP * (k + 1)], ps[:],
                    axis=mybir.AxisListType.X, op=mybir.AluOpType.min)

            nc.sync.dma_start(out_d[:], outt[:])
    nc.compile()
    return nc


def _morton_key(pts):
    rng = pts.max(0) - pts.min(0)
    q = ((pts - pts.min(0)) / (rng + 1e-9) * 1023).astype(np.uint64)

    def spread(x):
        x = x & np.uint64(0x3FF)
        x = (x | (x << np.uint64(16))) & np.uint64(0x30000FF)
        x = (x | (x << np.uint64(8))) & np.uint64(0x300F00F)
        x = (x | (x << np.uint64(4))) & np.uint64(0x30C30C3)
        x = (x | (x << np.uint64(2))) & np.uint64(0x9249249)
        return x

    return (spread(q[:, 0]) | (spread(q[:, 1]) << np.uint64(1))
            | (spread(q[:, 2]) << np.uint64(2)))


def _prep_core(X, Y):
    """Host prep for one (batch, direction): returns in_map plus the metadata
    needed to verify and assemble the result."""
    xo = np.argsort(X[:, AXIS], kind="stable")
    yo = np.argsort(Y[:, AXIS], kind="stable")
    Xs = X[xo]
    Ys = Y[yo]
    X2 = (Xs.astype(np.float64) ** 2).sum(1)
    Y2 = (Ys.astype(np.float64) ** 2).sum(1)
    zx = Xs[:, AXIS].astype(np.float64)
    zy = Ys[:, AXIS].astype(np.float64)

    # gap to nearest excluded Y along the sort axis, per query
    i = np.arange(M)
    c = i // SUB
    lo_pos = SUB * c - PAD          # first included Y position
    hi_pos = SUB * c + (BAND - PAD)  # first excluded upper position
    gap = np.full(M, np.inf)
    has_lo = lo_pos > 0
    gap[has_lo] = zx[has_lo] - zy[lo_pos[has_lo] - 1]
    has_hi = hi_pos < P
    gap[has_hi] = np.minimum(gap[has_hi], zy[hi_pos[has_hi]] - zx[has_hi])
    gap = np.maximum(gap, 0.0)

    # conservative NN-distance upper bound via Morton-order neighbors
    allpts = np.concatenate([Xs, Ys]).astype(np.float64)
    mk = _morton_key(allpts)
    inv = np.empty(2 * M, dtype=np.int64)
    inv[np.argsort(mk, kind="stable")] = np.arange(2 * M)
    y_rank = inv[M:]
    order_y = np.argsort(y_rank, kind="stable")
    sorted_ranks = y_rank[order_y]
    K = 16
    idx = np.searchsorted(sorted_ranks, inv[:M])
    cand = np.clip(idx[:, None] + np.arange(-K, K)[None, :], 0, M - 1)
    cands = order_y[cand]
    d2 = ((Xs[:, None, :].astype(np.float64) - Ys[cands].astype(np.float64)) ** 2).sum(-1)
    d_cap2 = d2.min(1)

    hard = np.flatnonzero(~(d_cap2 <= (gap * gap) * 0.98))
    if len(hard) > 128:
        score = np.sqrt(d_cap2[hard]) - gap[hard]
        hard = hard[np.argsort(-score)[:128]]
    over_idx = np.full(128, hard[0] if len(hard) else 0, dtype=np.int64)
    over_idx[:len(hard)] = hard

    # fp16 hi/lo decomposition of SCALE*X and SCALE*Y; device computes
    # SCALE^2 * (|y|^2 - 2 x.y) in fp32 PSUM via K=11 contraction rows:
    #   r0-2: -2*a_d * c_d     r3-5: -2*a_d * e_d     r6-8: -2*b_d * c_d
    #   r9:   1 * w_hi         r10:  1 * w_lo
    # where a+b ~ SCALE*x, c+e ~ SCALE*y, w_hi+w_lo ~ |SCALE*y|^2.
    Xss = (SCALE * Xs).astype(np.float64)
    Yss = (SCALE * Ys).astype(np.float64)
    a = Xss.astype(np.float16)
    bb = (Xss - a.astype(np.float64)).astype(np.float16)
    c = Yss.astype(np.float16)
    e = (Yss - c.astype(np.float64)).astype(np.float16)
    w = (Yss ** 2).sum(1)
    wh = w.astype(np.float16)
    wl = (w - wh.astype(np.float64)).astype(np.float16)

    wr = np.empty((KROWS, WT_W + RT_W), dtype=np.float16)
    wt = wr[:, :WT_W]
    rt = wr[:, WT_W:]

    na = (-2.0 * a.astype(np.float64)).astype(np.float16)  # exact: x2 of fp16
    nb = (-2.0 * bb.astype(np.float64)).astype(np.float16)
    wt[0:3, :M] = na.T
    wt[3:6, :M] = na.T
    wt[6:9, :M] = nb.T
    wt[9:11, :M] = 1.0
    wt[0:3, M:] = na[over_idx].T
    wt[3:6, M:] = na[over_idx].T
    wt[6:9, M:] = nb[over_idx].T
    wt[9:11, M:] = 1.0

    ccT = c.T
    eeT = e.T
    # edge-duplicate padding: repeats of the first/last sorted reference
    # point — real candidates, can never lower a min below the true min.
    for cols, sl in ((slice(0, PAD), 0), (slice(PAD + P, RT_W), P - 1)):
        rt[0:3, cols] = ccT[:, sl:sl + 1]
        rt[3:6, cols] = eeT[:, sl:sl + 1]
        rt[6:9, cols] = ccT[:, sl:sl + 1]
        rt[9, cols] = wh[sl]
        rt[10, cols] = wl[sl]
    rt[0:3, PAD:PAD + P] = ccT
    rt[3:6, PAD:PAD + P] = eeT
    rt[6:9, PAD:PAD + P] = ccT
    rt[9, PAD:PAD + P] = wh
    rt[10, PAD:PAD + P] = wl

    return {"wr": wr}, {
        "Xs": Xs.astype(np.float64), "Ys": Ys.astype(np.float64),
        "X2": X2, "Y2": Y2, "gap": gap, "hard": hard, "over_idx": over_idx,
    }


def _post_core(out, meta):
    """Combine device output into sum over queries of min-D (float64)."""
    inv_s2 = 1.0 / (SCALE * SCALE)
    band_min = out[:, :NBLK].T.reshape(M).astype(np.float64) * inv_s2
    dmin = band_min + meta["X2"]

    over_min = out[:, NBLK:].min(axis=1).astype(np.float64) * inv_s2
    over_d = over_min + meta["X2"][meta["over_idx"]]
    nhard = len(meta["hard"])
    if nhard:
        dmin[meta["hard"]] = over_d[:nhard]

    # soundness check for band-only points (device fp32 margin included)
    g2 = meta["gap"] * meta["gap"]
    ok = dmin <= g2 - 1e-3 - 1e-3 * np.abs(dmin)
    ok[meta["hard"]] = True
    bad = np.flatnonzero(~ok)
    if len(bad):
        Xb = meta["Xs"][bad]
        db = (meta["Y2"][None, :] - 2.0 * (Xb @ meta["Ys"].T)).min(axis=1)
        dmin[bad] = db + meta["X2"][bad]
    return dmin.sum()


def _install_axon_profile_hook():
    """Make trace=True work under axon when the image's antenv lacks
    axon_hooks: inject a shim module wired to the ctypes NTFF driver."""
    import sys
    import types
    try:
        from antenv.axon_hooks import get_axon_ntff_profile_hook  # noqa: F401
        return
    except ImportError:
        pass
    try:
        import antenv
        from trn_agent_boot.trn_boot import _ntff_profile_via_ctypes
        hook = _ntff_profile_via_ctypes("/opt/axon/libaxon_pjrt.so")
    except Exception:
        hook = None
    mod = types.ModuleType("antenv.axon_hooks")
    state = {"h": hook}
    mod.get_axon_ntff_profile_hook = lambda: state["h"]
    mod.set_axon_ntff_profile_hook = lambda h: state.__setitem__("h", h)
    sys.modules["antenv.axon_hooks"] = mod
    try:
        antenv.axon_hooks = mod
    except Exception:
        pass


def kernel(x_hat, points, likelihoods):
    from concourse.bass_utils import run_bass_kernel_spmd
    global LAST_RESULTS

    trace = bool(int(os.environ.get("CHAMFER_TRACE", "0")))
    if trace:
        _install_axon_profile_hook()

    if "nc" not in _CACHE:
        _CACHE["nc"] = _build_bass()
    nc = _CACHE["nc"]

    in_maps, metas = [], []
    for core in range(8):
        b, d = core // 2, core % 2
        X = x_hat[b] if d == 0 else points[b]
        Y = points[b] if d == 0 else x_hat[b]
        m, meta = _prep_core(np.asarray(X), np.asarray(Y))
        in_maps.append(m)
        metas.append(meta)

    res = run_bass_kernel_spmd(
        nc, in_maps, core_ids=list(range(8)), trace=trace,
    )
    LAST_RESULTS = res

    sums = [_post_core(res.results[c]["out"], metas[c]) for c in range(8)]
    cham_x = sum(sums[c] for c in range(8) if c % 2 == 0) / (B * M)
    cham_y = sum(sums[c] for c in range(8) if c % 2 == 1) / (B * P)
    rec = cham_x + cham_y

    lik = np.asarray(likelihoods, dtype=np.float64)
    bpp = np.log2(lik).sum() / (-(B * P))

    loss = bpp + LMBDA * rec
    return np.array([loss, bpp, rec], dtype=np.float32)



# revision 6
# speedup vs baseline: 1.2077x; 1.2077x over previous
"""Chamfer rate-distortion loss on 8 TRN2 NeuronCores — v2.

Layout: 8 cores = 4 batches x 2 chamfer directions. Each core computes, for
its (batch, direction), the per-point nearest-neighbor squared distance of
8192 query points X against 8192 reference points Y.

v2 design (vs v1 baseline at ~54us):
  - PSUM holds SCALE^2 * d(x,y)^2 directly: the K=13 fp16 hi/lo matmul now
    also folds |x|^2 in (stationary rows x2_hi/x2_lo vs moving 1.0), so PSUM
    values are >= 0 and both reduce lanes read them unmodified.
  - BAND=256 (PAD=112) sorted bands, 16 PSUM tiles of [128, 2banks, 2, 256]
    (two query blocks share one 2KB bank).
  - Dual reduce lanes drain PSUM in parallel:
      * DVE lane: exact tensor_reduce(min) on ~9 tiles.
      * ScalarE lane: softmin via activation(Exp, scale=-16) with accum_out,
        i.e. S_q = sum_j exp(-16384 * d_qj); host recovers
        min ~= -ln(S)/16384 + corr, where corr is calibrated per-core from
        two sub-groups computed by BOTH lanes (removes the softmin bias;
        residual ~2e-5 per point, rec rel err ~2.5e-3 measured).
  - Queries whose NN may fall outside their band (Morton-certified on host,
    need > PAD) go to 12 overflow chunks of 32; each chunk scans a host-
    chosen 1024-wide rank window (gathered into the input), not the full Y.
  - Far-point padding at band edges (never a min; exp underflows to 0).
  - Head-first DMA ordering so the first matmul starts ~1us after DMA begins;
    output DMA split in two; dummy Exp at t0 prefetches the ACT table set.

Soundness: coverage comes from the host-side Morton certificate (d_cap from
32 Morton-order candidates): need<=PAD queries provably have their NN inside
the band; hard queries are covered by their overflow window or recomputed
exactly on host (spill + softmin underflows, ~100-200 of 8192 per core).
"""

import os

import numpy as np

B, M, P = 4, 8192, 8192
SUB = 32
PAD = 112
BAND = SUB + 2 * PAD          # 256
NTILE = 16                    # band tiles; each = 4 blocks of 128 queries
NOFCH = 12                    # overflow chunks of 32 hard queries
OFW = 1024                    # overflow window width (2 x 512)
NOFT = NOFCH // 4             # overflow tiles
KROWS = 13
SCALE = 32.0
S2 = SCALE * SCALE            # 1024
ACT_SCALE = -16.0             # exp(-16 * PSUM) = exp(-16384 * d)
SPRIME = -ACT_SCALE * S2      # 16384
LMBDA = 5.0

WT_W = M + NOFCH * 32                 # 8576: band stationary | OF stationary
RT_W = P + 2 * PAD                    # 8416: far | sorted Y | far
OF_W = NOFCH * OFW                    # 12288
TOT_W = WT_W + RT_W + OF_W            # 29280
RT0 = WT_W
OF0 = WT_W + RT_W

# lane assignment: which band tiles the ScalarE softmin lane owns.
# Overflow tiles are always DVE-exact: hard queries have large d and their
# exp(-16384 d) underflows to 0, so softmin cannot serve them.
SOFT_TILES = (1, 2, 3, 5, 7, 9, 11, 13, 14)
CAL_TILES = (1, 5, 9, 13)     # group 0 of these ALSO gets a DVE exact reduce
NOUT = 72                     # 64 band | 64..67 calib | 68..70 OF | pad

_CACHE = {}
LAST_RESULTS = None


def _build_bass():
    import concourse.tile as tile
    from concourse import bacc, mybir

    nc = bacc.Bacc(None, target_bir_lowering=False, debug=False)
    f32 = mybir.dt.float32
    f16 = mybir.dt.float16

    wr_d = nc.dram_tensor("wr", [KROWS, TOT_W], f16, kind="ExternalInput")
    out_d = nc.dram_tensor("out", [128, NOUT], f32, kind="ExternalOutput")

    with tile.TileContext(nc) as tc:
        with (
            tc.tile_pool(name="const", bufs=1) as cpool,
            tc.tile_pool(name="outp", bufs=1) as opool,
            tc.tile_pool(name="scr", bufs=2) as spool,
            tc.tile_pool(name="psum", bufs=3, space="PSUM") as ppool,
        ):
            wr = cpool.tile([KROWS, TOT_W], f16)
            # head pieces first so the PE can start ~1us after DMA begins
            nc.sync.dma_start(wr[:, 0:512], wr_d[:, 0:512])
            nc.sync.dma_start(wr[:, RT0:RT0 + 768], wr_d[:, RT0:RT0 + 768])
            nc.sync.dma_start(wr[:, 512:4608], wr_d[:, 512:4608])
            nc.sync.dma_start(wr[:, RT0 + 768:RT0 + 4864],
                              wr_d[:, RT0 + 768:RT0 + 4864])
            nc.sync.dma_start(wr[:, 4608:RT0], wr_d[:, 4608:RT0])
            nc.sync.dma_start(wr[:, RT0 + 4864:OF0], wr_d[:, RT0 + 4864:OF0])
            nc.sync.dma_start(wr[:, OF0:], wr_d[:, OF0:])

            outt = opool.tile([128, NOUT], f32)

            # dummy Exp pulls the ACT table load off the critical path
            warm = spool.tile([128, 1], f32, tag="warm")
            nc.vector.memset(warm[:], 0.0)
            nc.scalar.activation(warm[:], warm[:],
                                 mybir.ActivationFunctionType.Exp, scale=1.0)

            for k in range(NTILE):
                ps = ppool.tile([128, 2, 2, BAND], f32, tag="ps")
                for g in range(4):
                    for s in range(4):
                        u = 16 * k + 4 * g + s
                        nc.tensor.matmul(
                            ps[32 * s:32 * s + 32, g // 2, g % 2, :],
                            wr[:, SUB * u:SUB * u + SUB],
                            wr[:, RT0 + SUB * u:RT0 + SUB * u + BAND],
                            start=True, stop=True,
                            tile_position=(0, 32 * s),
                        )
                if k in SOFT_TILES:
                    scr = spool.tile([128, 2, 2, BAND], f32, tag="scr")
                    for g in range(4):
                        nc.scalar.activation(
                            scr[:, g // 2, g % 2, :],
                            ps[:, g // 2, g % 2, :],
                            mybir.ActivationFunctionType.Exp,
                            scale=ACT_SCALE,
                            accum_out=outt[:, 4 * k + g:4 * k + g + 1],
                        )
                    if k in CAL_TILES:
                        j = CAL_TILES.index(k)
                        nc.vector.tensor_reduce(
                            outt[:, 64 + j:65 + j], ps[:, 0, 0, :],
                            axis=mybir.AxisListType.X, op=mybir.AluOpType.min)
                else:
                    nc.vector.tensor_reduce(
                        outt[:, 4 * k:4 * (k + 1)], ps[:],
                        axis=mybir.AxisListType.X, op=mybir.AluOpType.min)
                if k == 7:
                    nc.sync.dma_start(out_d[:, 0:32], outt[:, 0:32])

            for t in range(NOFT):
                ps = ppool.tile([128, 2, 2, BAND], f32, tag="ps")
                for c in range(4):
                    cc = 4 * t + c
                    for h in range(2):
                        nc.tensor.matmul(
                            ps[32 * c:32 * c + 32, h, :, :],
                            wr[:, M + 32 * cc:M + 32 * cc + 32],
                            wr[:, OF0 + OFW * cc + 512 * h:
                                  OF0 + OFW * cc + 512 * h + 512],
                            start=True, stop=True,
                            tile_position=(0, 32 * c),
                        )
                nc.vector.tensor_reduce(
                    outt[:, 68 + t:69 + t], ps[:],
                    axis=mybir.AxisListType.XYZ, op=mybir.AluOpType.min)

            nc.sync.dma_start(out_d[:, 32:NOUT], outt[:, 32:NOUT])
    nc.compile()
    return nc


def _morton_key(pts):
    rng = pts.max(0) - pts.min(0)
    q = ((pts - pts.min(0)) / (rng + 1e-9) * 1023).astype(np.uint64)

    def spread(x):
        x = x & np.uint64(0x3FF)
        x = (x | (x << np.uint64(16))) & np.uint64(0x30000FF)
        x = (x | (x << np.uint64(8))) & np.uint64(0x300F00F)
        x = (x | (x << np.uint64(4))) & np.uint64(0x30C30C3)
        x = (x | (x << np.uint64(2))) & np.uint64(0x9249249)
        return x

    return (spread(q[:, 0]) | (spread(q[:, 1]) << np.uint64(1))
            | (spread(q[:, 2]) << np.uint64(2)))


def _prep_core(X, Y):
    """Host prep for one (batch, direction)."""
    X64 = X.astype(np.float64)
    Y64 = Y.astype(np.float64)

    # Morton d_cap (axis-independent NN upper bound from 32 candidates)
    allpts = np.concatenate([X64, Y64])
    mk = _morton_key(allpts)
    inv = np.empty(2 * M, dtype=np.int64)
    inv[np.argsort(mk, kind="stable")] = np.arange(2 * M)
    y_rank = inv[M:]
    order_y = np.argsort(y_rank, kind="stable")
    sorted_ranks = y_rank[order_y]
    idx = np.searchsorted(sorted_ranks, inv[:M])
    cand = np.clip(idx[:, None] + np.arange(-16, 16)[None, :], 0, M - 1)
    cands = order_y[cand]
    d_cap2 = ((X64[:, None, :] - Y64[cands]) ** 2).sum(-1).min(1)
    d_cap = np.sqrt(d_cap2 / 0.98)

    # choose the sort axis with the fewest hard queries
    i = np.arange(M)
    best = None
    for axis in range(3):
        xo = np.argsort(X[:, axis], kind="stable")
        yo = np.argsort(Y[:, axis], kind="stable")
        zx = X64[xo, axis]
        zy = Y64[yo, axis]
        dc = d_cap[xo]
        lo_idx = np.searchsorted(zy, zx - dc)
        hi_idx = np.searchsorted(zy, zx + dc)
        cch = i // SUB
        need = np.maximum(np.maximum(SUB * cch - lo_idx,
                                     hi_idx - (SUB * cch + SUB)), 0)
        nhard = int((need > PAD).sum())
        if best is None or nhard < best[0]:
            best = (nhard, axis, xo, yo, lo_idx, hi_idx, need)
    _, axis, xo, yo, lo_idx, hi_idx, need = best

    Xs = X64[xo]
    Ys = Y64[yo]
    X2 = (Xs ** 2).sum(1)
    Y2 = (Ys ** 2).sum(1)

    # far pad point: beyond data range along the sort axis, never a min
    zfar = np.abs(np.concatenate([Xs[:, axis], Ys[:, axis]])).max() + 2.0
    ypad = np.zeros(3)
    ypad[axis] = zfar
    Yx = np.vstack([Ys, ypad])    # index P = pad

    # fp16 hi/lo decomposition
    Xss = SCALE * Xs
    Yss = SCALE * Yx
    a = Xss.astype(np.float16)
    bb = (Xss - a.astype(np.float64)).astype(np.float16)
    c = Yss.astype(np.float16)
    e = (Yss - c.astype(np.float64)).astype(np.float16)
    w = (Yss ** 2).sum(1)
    wh = w.astype(np.float16)
    wl = (w - wh.astype(np.float64)).astype(np.float16)
    v = (Xss ** 2).sum(1)
    vh = v.astype(np.float16)
    vl = (v - vh.astype(np.float64)).astype(np.float16)
    na = (-2.0 * a.astype(np.float64)).astype(np.float16)
    nb = (-2.0 * bb.astype(np.float64)).astype(np.float16)

    # hard queries -> overflow chunks with per-chunk 1024 windows
    hard = np.flatnonzero(need > PAD)
    of_idx = np.zeros(NOFCH * 32, dtype=np.int64)     # query per slot
    of_valid = np.zeros(NOFCH * 32, dtype=bool)
    w0s = np.zeros(NOFCH, dtype=np.int64)
    spill = list(hard[NOFCH * 32:])
    for ccn in range(NOFCH):
        grp = hard[32 * ccn:32 * ccn + 32]
        if len(grp) == 0:
            continue
        loi, hii = lo_idx[grp], hi_idx[grp]
        # pick w0 among candidate starts maximizing covered members
        cands_w0 = np.clip(np.concatenate([loi, hii - OFW]), 0, P - OFW)
        covn = ((loi[None, :] >= cands_w0[:, None])
                & (hii[None, :] <= cands_w0[:, None] + OFW)).sum(1)
        w0 = int(cands_w0[covn.argmax()])
        w0s[ccn] = w0
        cov = (loi >= w0) & (hii <= w0 + OFW)
        nslot = len(grp)
        of_idx[32 * ccn:32 * ccn + nslot] = grp
        of_valid[32 * ccn:32 * ccn + nslot] = cov
        spill.extend(grp[~cov])
        if nslot < 32:
            of_idx[32 * ccn + nslot:32 * ccn + 32] = grp[0]

    # device input
    wr = np.zeros((KROWS, TOT_W), dtype=np.float16)
    wt = wr[:, :WT_W]
    rt = wr[:, RT0:RT0 + RT_W]
    ofr = wr[:, OF0:]

    wt[0:3, :M] = na.T
    wt[3:6, :M] = na.T
    wt[6:9, :M] = nb.T
    wt[9:11, :M] = 1.0
    wt[11, :M] = vh
    wt[12, :M] = vl
    wt[0:3, M:] = na[of_idx].T
    wt[3:6, M:] = na[of_idx].T
    wt[6:9, M:] = nb[of_idx].T
    wt[9:11, M:] = 1.0
    wt[11, M:] = vh[of_idx]
    wt[12, M:] = vl[of_idx]

    # rt: [0:PAD]=far pad, [PAD:PAD+P]=sorted Y, [PAD+P:]=far pad
    ridx = np.full(RT_W, P, dtype=np.int64)
    ridx[PAD:PAD + P] = np.arange(P)
    rt[0:3, :] = c[ridx].T
    rt[3:6, :] = e[ridx].T
    rt[6:9, :] = c[ridx].T
    rt[9, :] = wh[ridx]
    rt[10, :] = wl[ridx]
    rt[11:13, :] = 1.0

    # overflow windows (real Y columns, no pad needed)
    oidx = (w0s[:, None] + np.arange(OFW)[None, :]).reshape(-1)
    ofr[0:3, :] = c[oidx].T
    ofr[3:6, :] = e[oidx].T
    ofr[6:9, :] = c[oidx].T
    ofr[9, :] = wh[oidx]
    ofr[10, :] = wl[oidx]
    ofr[11:13, :] = 1.0

    return {"wr": wr}, {
        "Xs": Xs, "Ys": Ys, "X2": X2, "Y2": Y2,
        "hard": hard, "of_idx": of_idx, "of_valid": of_valid,
        "d_cap2": d_cap2[xo],
        "spill": np.array(sorted(set(int(s) for s in spill)), dtype=np.int64),
    }


def _post_core(out, meta):
    """Combine device output into sum over queries of min-D (float64)."""
    out = out.astype(np.float64)
    est = np.empty(M)
    softq = np.zeros(M, dtype=bool)
    for k in range(NTILE):
        vals = out[:, 4 * k:4 * k + 4].T.reshape(512)   # queries 512k..+511
        sl = slice(512 * k, 512 * k + 512)
        if k in SOFT_TILES:
            with np.errstate(divide="ignore"):
                est[sl] = np.where(vals > 0.0,
                                   -np.log(np.maximum(vals, 1e-300)) / SPRIME,
                                   np.inf)
            softq[sl] = True
        else:
            est[sl] = vals / S2

    # softmin bias calibration from the doubly-computed sub-groups
    diffs = []
    for j, k in enumerate(CAL_TILES):
        exact = out[:, 64 + j] / S2                     # queries 512k..+127
        soft = est[512 * k:512 * k + 128]
        ok = np.isfinite(soft)
        diffs.append((exact - soft)[ok])
    dall = np.concatenate(diffs)
    corr = dall.mean() if len(dall) else 0.0
    est[softq] += corr

    # overflow results (always DVE-exact min over the chunk window)
    for t in range(NOFT):
        vals = out[:, 68 + t] / S2
        for p in range(128):
            slot = 128 * t + p
            if not meta["of_valid"][slot]:
                continue
            q = meta["of_idx"][slot]
            if vals[p] < est[q]:
                est[q] = vals[p]

    # host-exact fixes: spill + softmin underflows + d_cap sanity violations
    # (est is always a restricted min >= true; d_cap2 >= true NN distance, and
    #  any correctly-covered query must satisfy est <= d_cap2 up to noise).
    fix = set(int(q) for q in meta["spill"])
    fix.update(int(q) for q in np.flatnonzero(~np.isfinite(est)))
    fix.update(int(q) for q in
               np.flatnonzero(est > meta["d_cap2"] / 0.98 + 2e-4))
    if fix:
        qq = np.array(sorted(fix), dtype=np.int64)
        D = (meta["Y2"][None, :] - 2.0 * (meta["Xs"][qq] @ meta["Ys"].T))
        est[qq] = D.min(1) + meta["X2"][qq]
    return est.sum()


def _install_axon_profile_hook():
    import sys
    import types
    try:
        from antenv.axon_hooks import get_axon_ntff_profile_hook  # noqa: F401
        return
    except ImportError:
        pass
    try:
        import antenv
        from trn_agent_boot.trn_boot import _ntff_profile_via_ctypes
        hook = _ntff_profile_via_ctypes("/opt/axon/libaxon_pjrt.so")
    except Exception:
        hook = None
    mod = types.ModuleType("antenv.axon_hooks")
    state = {"h": hook}
    mod.get_axon_ntff_profile_hook = lambda: state["h"]
    mod.set_axon_ntff_profile_hook = lambda h: state.__setitem__("h", h)
    sys.modules["antenv.axon_hooks"] = mod
    try:
        antenv.axon_hooks = mod
    except Exception:
        pass


def kernel(x_hat, points, likelihoods):
    from concourse.bass_utils import run_bass_kernel_spmd
    global LAST_RESULTS

    trace = bool(int(os.environ.get("CHAMFER_TRACE", "0")))
    if trace:
        _install_axon_profile_hook()

    if "nc" not in _CACHE:
        _CACHE["nc"] = _build_bass()
    nc = _CACHE["nc"]

    in_maps, metas = [], []
    for core in range(8):
        b, d = core // 2, core % 2
        X = x_hat[b] if d == 0 else points[b]
        Y = points[b] if d == 0 else x_hat[b]
        m, meta = _prep_core(np.asarray(X), np.asarray(Y))
        in_maps.append(m)
        metas.append(meta)

    res = run_bass_kernel_spmd(
        nc, in_maps, core_ids=list(range(8)), trace=trace,
    )
    LAST_RESULTS = res

    sums = [_post_core(res.results[c]["out"], metas[c]) for c in range(8)]
    cham_x = sum(sums[c] for c in range(8) if c % 2 == 0) / (B * M)
    cham_y = sum(sums[c] for c in range(8) if c % 2 == 1) / (B * P)
    rec = cham_x + cham_y

    lik = np.asarray(likelihoods, dtype=np.float64)
    bpp = np.log2(lik).sum() / (-(B * P))

    loss = bpp + LMBDA * rec
    return np.array([loss, bpp, rec], dtype=np.float32)


# revision 9
# speedup vs baseline: 1.3667x; 1.1317x over previous
"""Chamfer rate-distortion loss on 8 TRN2 NeuronCores — v2.

Layout: 8 cores = 4 batches x 2 chamfer directions. Each core computes, for
its (batch, direction), the per-point nearest-neighbor squared distance of
8192 query points X against 8192 reference points Y.

v2 design (vs v1 baseline at ~54us):
  - PSUM holds SCALE^2 * d(x,y)^2 directly: the K=13 fp16 hi/lo matmul now
    also folds |x|^2 in (stationary rows x2_hi/x2_lo vs moving 1.0), so PSUM
    values are >= 0 and both reduce lanes read them unmodified.
  - BAND=256 (PAD=112) sorted bands, 16 PSUM tiles of [128, 2banks, 2, 256]
    (two query blocks share one 2KB bank).
  - Dual reduce lanes drain PSUM in parallel:
      * DVE lane: exact tensor_reduce(min) on ~9 tiles.
      * ScalarE lane: softmin via activation(Exp, scale=-16) with accum_out,
        i.e. S_q = sum_j exp(-16384 * d_qj); host recovers
        min ~= -ln(S)/16384 + corr, where corr is calibrated per-core from
        two sub-groups computed by BOTH lanes (removes the softmin bias;
        residual ~2e-5 per point, rec rel err ~2.5e-3 measured).
  - Queries whose NN may fall outside their band (Morton-certified on host,
    need > PAD) go to 12 overflow chunks of 32; each chunk scans a host-
    chosen 1024-wide rank window (gathered into the input), not the full Y.
  - Far-point padding at band edges (never a min; exp underflows to 0).
  - Head-first DMA ordering so the first matmul starts ~1us after DMA begins;
    output DMA split in two; dummy Exp at t0 prefetches the ACT table set.

Soundness: coverage comes from the host-side Morton certificate (d_cap from
32 Morton-order candidates): need<=PAD queries provably have their NN inside
the band; hard queries are covered by their overflow window or recomputed
exactly on host (spill + softmin underflows, ~100-200 of 8192 per core).
"""

import os

import numpy as np

B, M, P = 4, 8192, 8192
SUB = 32
PAD = 112
BAND = SUB + 2 * PAD          # 256
NTILE = 16                    # band tiles; each = 4 blocks of 128 queries
NOFCH = 12                    # overflow chunks of 32 hard queries
OFW = 1024                    # overflow window width (2 x 512)
NOFT = NOFCH // 4             # overflow tiles
KROWS = 13
SCALE = 32.0
S2 = SCALE * SCALE            # 1024
ACT_SCALE = -16.0             # exp(-16 * PSUM) = exp(-16384 * d)
SPRIME = -ACT_SCALE * S2      # 16384
LMBDA = 5.0

WT_W = M + NOFCH * 32                 # 8576: band stationary | OF stationary
RT_W = P + 2 * PAD                    # 8416: far | sorted Y | far
OF_W = NOFCH * OFW                    # 12288
TOT_W = WT_W + RT_W + OF_W            # 29280
RT0 = WT_W
OF0 = WT_W + RT_W

# lane assignment: which band tiles the ScalarE softmin lane owns.
# Overflow tiles are always DVE-exact: hard queries have large d and their
# exp(-16384 d) underflows to 0, so softmin cannot serve them.
SOFT_TILES = (3, 5, 7, 11, 13, 15)
CAL_TILES = (3, 7, 13)        # group 0 of these ALSO gets a DVE exact reduce
NCAL = len(CAL_TILES)
OFCOL = 64 + NCAL             # overflow out cols start here
NOUT = 72                     # 64 band | calib | OF | pad

_CACHE = {}
LAST_RESULTS = None


def _build_bass():
    import concourse.tile as tile
    from concourse import bacc, mybir

    nc = bacc.Bacc(None, target_bir_lowering=False, debug=False)
    f32 = mybir.dt.float32
    f16 = mybir.dt.float16

    wr_d = nc.dram_tensor("wr", [KROWS, TOT_W], f16, kind="ExternalInput")
    out_d = nc.dram_tensor("out", [128, NOUT], f32, kind="ExternalOutput")

    with tile.TileContext(nc) as tc:
        with (
            tc.tile_pool(name="const", bufs=1) as cpool,
            tc.tile_pool(name="outp", bufs=1) as opool,
            tc.tile_pool(name="scr", bufs=2) as spool,
            tc.tile_pool(name="psum_d", bufs=2, space="PSUM") as ppool_d,
            tc.tile_pool(name="psum_s", bufs=2, space="PSUM") as ppool_s,
        ):
            wr = cpool.tile([KROWS, TOT_W], f16)
            # head pieces first (sync queue); bulk via the scalar queue so
            # the issues overlap and the PE can start ~2us after DMA begins
            nc.sync.dma_start(wr[:, 0:512], wr_d[:, 0:512])
            nc.sync.dma_start(wr[:, RT0:RT0 + 768], wr_d[:, RT0:RT0 + 768])
            nc.scalar.dma_start(wr[:, 512:4608], wr_d[:, 512:4608])
            nc.sync.dma_start(wr[:, RT0 + 768:RT0 + 4864],
                              wr_d[:, RT0 + 768:RT0 + 4864])
            nc.scalar.dma_start(wr[:, 4608:RT0], wr_d[:, 4608:RT0])
            nc.sync.dma_start(wr[:, RT0 + 4864:OF0], wr_d[:, RT0 + 4864:OF0])
            nc.scalar.dma_start(wr[:, OF0:], wr_d[:, OF0:])

            outt = opool.tile([128, NOUT], f32)

            # dummy Exp pulls the ACT table load off the critical path
            warm = spool.tile([128, 1], f32, tag="warm")
            nc.vector.memset(warm[:], 0.0)
            nc.scalar.activation(warm[:], warm[:],
                                 mybir.ActivationFunctionType.Exp, scale=1.0)

            for k in range(NTILE):
                soft = k in SOFT_TILES
                pool = ppool_s if soft else ppool_d
                ps = pool.tile([128, 2, 2, BAND], f32, tag="ps")
                for g in range(4):
                    for s in range(4):
                        u = 16 * k + 4 * g + s
                        nc.tensor.matmul(
                            ps[32 * s:32 * s + 32, g // 2, g % 2, :],
                            wr[:, SUB * u:SUB * u + SUB],
                            wr[:, RT0 + SUB * u:RT0 + SUB * u + BAND],
                            start=True, stop=True,
                            tile_position=(0, 32 * s),
                        )
                if soft:
                    scr = spool.tile([128, 2, 2, BAND], f32, tag="scr")
                    for g in range(4):
                        nc.scalar.activation(
                            scr[:, g // 2, g % 2, :],
                            ps[:, g // 2, g % 2, :],
                            mybir.ActivationFunctionType.Exp,
                            scale=ACT_SCALE,
                            accum_out=outt[:, 4 * k + g:4 * k + g + 1],
                        )
                    if k in CAL_TILES:
                        j = CAL_TILES.index(k)
                        nc.vector.tensor_reduce(
                            outt[:, 64 + j:65 + j], ps[:, 0, 0, :],
                            axis=mybir.AxisListType.X, op=mybir.AluOpType.min)
                else:
                    nc.vector.tensor_reduce(
                        outt[:, 4 * k:4 * (k + 1)], ps[:],
                        axis=mybir.AxisListType.X, op=mybir.AluOpType.min)
                if k == 7:
                    nc.sync.dma_start(out_d[:, 0:32], outt[:, 0:32])

            for t in range(NOFT):
                ps = ppool_d.tile([128, 2, 2, BAND], f32, tag="ps")
                for c in range(4):
                    cc = 4 * t + c
                    for h in range(2):
                        nc.tensor.matmul(
                            ps[32 * c:32 * c + 32, h, :, :],
                            wr[:, M + 32 * cc:M + 32 * cc + 32],
                            wr[:, OF0 + OFW * cc + 512 * h:
                                  OF0 + OFW * cc + 512 * h + 512],
                            start=True, stop=True,
                            tile_position=(0, 32 * c),
                        )
                nc.vector.tensor_reduce(
                    outt[:, OFCOL + t:OFCOL + t + 1], ps[:],
                    axis=mybir.AxisListType.XYZ, op=mybir.AluOpType.min)
                if t == 0:
                    nc.sync.dma_start(out_d[:, 32:64], outt[:, 32:64])

            nc.sync.dma_start(out_d[:, 64:NOUT], outt[:, 64:NOUT])
    nc.compile()
    return nc


def _morton_key(pts):
    rng = pts.max(0) - pts.min(0)
    q = ((pts - pts.min(0)) / (rng + 1e-9) * 1023).astype(np.uint64)

    def spread(x):
        x = x & np.uint64(0x3FF)
        x = (x | (x << np.uint64(16))) & np.uint64(0x30000FF)
        x = (x | (x << np.uint64(8))) & np.uint64(0x300F00F)
        x = (x | (x << np.uint64(4))) & np.uint64(0x30C30C3)
        x = (x | (x << np.uint64(2))) & np.uint64(0x9249249)
        return x

    return (spread(q[:, 0]) | (spread(q[:, 1]) << np.uint64(1))
            | (spread(q[:, 2]) << np.uint64(2)))


def _prep_core(X, Y):
    """Host prep for one (batch, direction)."""
    X64 = X.astype(np.float64)
    Y64 = Y.astype(np.float64)

    # Morton d_cap (axis-independent NN upper bound from 32 candidates)
    allpts = np.concatenate([X64, Y64])
    mk = _morton_key(allpts)
    inv = np.empty(2 * M, dtype=np.int64)
    inv[np.argsort(mk, kind="stable")] = np.arange(2 * M)
    y_rank = inv[M:]
    order_y = np.argsort(y_rank, kind="stable")
    sorted_ranks = y_rank[order_y]
    idx = np.searchsorted(sorted_ranks, inv[:M])
    cand = np.clip(idx[:, None] + np.arange(-16, 16)[None, :], 0, M - 1)
    cands = order_y[cand]
    d_cap2 = ((X64[:, None, :] - Y64[cands]) ** 2).sum(-1).min(1)
    d_cap = np.sqrt(d_cap2 / 0.98)

    # choose the sort axis with the fewest hard queries
    i = np.arange(M)
    best = None
    for axis in range(3):
        xo = np.argsort(X[:, axis], kind="stable")
        yo = np.argsort(Y[:, axis], kind="stable")
        zx = X64[xo, axis]
        zy = Y64[yo, axis]
        dc = d_cap[xo]
        lo_idx = np.searchsorted(zy, zx - dc)
        hi_idx = np.searchsorted(zy, zx + dc)
        cch = i // SUB
        need = np.maximum(np.maximum(SUB * cch - lo_idx,
                                     hi_idx - (SUB * cch + SUB)), 0)
        nhard = int((need > PAD).sum())
        if best is None or nhard < best[0]:
            best = (nhard, axis, xo, yo, lo_idx, hi_idx, need)
    _, axis, xo, yo, lo_idx, hi_idx, need = best

    Xs = X64[xo]
    Ys = Y64[yo]
    X2 = (Xs ** 2).sum(1)
    Y2 = (Ys ** 2).sum(1)

    # far pad point: beyond data range along the sort axis, never a min
    zfar = np.abs(np.concatenate([Xs[:, axis], Ys[:, axis]])).max() + 2.0
    ypad = np.zeros(3)
    ypad[axis] = zfar
    Yx = np.vstack([Ys, ypad])    # index P = pad

    # fp16 hi/lo decomposition
    Xss = SCALE * Xs
    Yss = SCALE * Yx
    a = Xss.astype(np.float16)
    bb = (Xss - a.astype(np.float64)).astype(np.float16)
    c = Yss.astype(np.float16)
    e = (Yss - c.astype(np.float64)).astype(np.float16)
    w = (Yss ** 2).sum(1)
    wh = w.astype(np.float16)
    wl = (w - wh.astype(np.float64)).astype(np.float16)
    v = (Xss ** 2).sum(1)
    vh = v.astype(np.float16)
    vl = (v - vh.astype(np.float64)).astype(np.float16)
    na = (-2.0 * a.astype(np.float64)).astype(np.float16)
    nb = (-2.0 * bb.astype(np.float64)).astype(np.float16)

    # hard queries -> overflow chunks with per-chunk 1024 windows
    hard = np.flatnonzero(need > PAD)
    of_idx = np.zeros(NOFCH * 32, dtype=np.int64)     # query per slot
    of_valid = np.zeros(NOFCH * 32, dtype=bool)
    w0s = np.zeros(NOFCH, dtype=np.int64)
    spill = list(hard[NOFCH * 32:])
    for ccn in range(NOFCH):
        grp = hard[32 * ccn:32 * ccn + 32]
        if len(grp) == 0:
            continue
        loi, hii = lo_idx[grp], hi_idx[grp]
        # pick w0 among candidate starts maximizing covered members
        cands_w0 = np.clip(np.concatenate([loi, hii - OFW]), 0, P - OFW)
        covn = ((loi[None, :] >= cands_w0[:, None])
                & (hii[None, :] <= cands_w0[:, None] + OFW)).sum(1)
        w0 = int(cands_w0[covn.argmax()])
        w0s[ccn] = w0
        cov = (loi >= w0) & (hii <= w0 + OFW)
        nslot = len(grp)
        of_idx[32 * ccn:32 * ccn + nslot] = grp
        of_valid[32 * ccn:32 * ccn + nslot] = cov
        spill.extend(grp[~cov])
        if nslot < 32:
            of_idx[32 * ccn + nslot:32 * ccn + 32] = grp[0]

    # device input
    wr = np.zeros((KROWS, TOT_W), dtype=np.float16)
    wt = wr[:, :WT_W]
    rt = wr[:, RT0:RT0 + RT_W]
    ofr = wr[:, OF0:]

    wt[0:3, :M] = na.T
    wt[3:6, :M] = na.T
    wt[6:9, :M] = nb.T
    wt[9:11, :M] = 1.0
    wt[11, :M] = vh
    wt[12, :M] = vl
    wt[0:3, M:] = na[of_idx].T
    wt[3:6, M:] = na[of_idx].T
    wt[6:9, M:] = nb[of_idx].T
    wt[9:11, M:] = 1.0
    wt[11, M:] = vh[of_idx]
    wt[12, M:] = vl[of_idx]

    # rt: [0:PAD]=far pad, [PAD:PAD+P]=sorted Y, [PAD+P:]=far pad
    ridx = np.full(RT_W, P, dtype=np.int64)
    ridx[PAD:PAD + P] = np.arange(P)
    rt[0:3, :] = c[ridx].T
    rt[3:6, :] = e[ridx].T
    rt[6:9, :] = c[ridx].T
    rt[9, :] = wh[ridx]
    rt[10, :] = wl[ridx]
    rt[11:13, :] = 1.0

    # overflow windows (real Y columns, no pad needed)
    oidx = (w0s[:, None] + np.arange(OFW)[None, :]).reshape(-1)
    ofr[0:3, :] = c[oidx].T
    ofr[3:6, :] = e[oidx].T
    ofr[6:9, :] = c[oidx].T
    ofr[9, :] = wh[oidx]
    ofr[10, :] = wl[oidx]
    ofr[11:13, :] = 1.0

    return {"wr": wr}, {
        "Xs": Xs, "Ys": Ys, "X2": X2, "Y2": Y2,
        "hard": hard, "of_idx": of_idx, "of_valid": of_valid,
        "d_cap2": d_cap2[xo],
        "spill": np.array(sorted(set(int(s) for s in spill)), dtype=np.int64),
    }


def _post_core(out, meta):
    """Combine device output into sum over queries of min-D (float64)."""
    out = out.astype(np.float64)
    est = np.empty(M)
    softq = np.zeros(M, dtype=bool)
    for k in range(NTILE):
        vals = out[:, 4 * k:4 * k + 4].T.reshape(512)   # queries 512k..+511
        sl = slice(512 * k, 512 * k + 512)
        if k in SOFT_TILES:
            with np.errstate(divide="ignore"):
                est[sl] = np.where(vals > 0.0,
                                   -np.log(np.maximum(vals, 1e-300)) / SPRIME,
                                   np.inf)
            softq[sl] = True
        else:
            est[sl] = vals / S2

    # softmin bias calibration from the doubly-computed sub-groups
    diffs = []
    for j, k in enumerate(CAL_TILES):
        exact = out[:, 64 + j] / S2                     # queries 512k..+127
        soft = est[512 * k:512 * k + 128]
        ok = np.isfinite(soft)
        diffs.append((exact - soft)[ok])
    dall = np.concatenate(diffs)
    corr = dall.mean() if len(dall) else 0.0
    est[softq] += corr

    # overflow results (always DVE-exact min over the chunk window)
    for t in range(NOFT):
        vals = out[:, OFCOL + t] / S2
        for p in range(128):
            slot = 128 * t + p
            if not meta["of_valid"][slot]:
                continue
            q = meta["of_idx"][slot]
            if vals[p] < est[q]:
                est[q] = vals[p]

    # host-exact fixes: spill + softmin underflows + d_cap sanity violations
    # (est is always a restricted min >= true; d_cap2 >= true NN distance, and
    #  any correctly-covered query must satisfy est <= d_cap2 up to noise).
    fix = set(int(q) for q in meta["spill"])
    fix.update(int(q) for q in np.flatnonzero(~np.isfinite(est)))
    fix.update(int(q) for q in
               np.flatnonzero(est > meta["d_cap2"] / 0.98 + 2e-4))
    if fix:
        qq = np.array(sorted(fix), dtype=np.int64)
        D = (meta["Y2"][None, :] - 2.0 * (meta["Xs"][qq] @ meta["Ys"].T))
        est[qq] = D.min(1) + meta["X2"][qq]
    return est.sum()


def _install_axon_profile_hook():
    import sys
    import types
    try:
        from antenv.axon_hooks import get_axon_ntff_profile_hook  # noqa: F401
        return
    except ImportError:
        pass
    try:
        import antenv
        from trn_agent_boot.trn_boot import _ntff_profile_via_ctypes
        hook = _ntff_profile_via_ctypes("/opt/axon/libaxon_pjrt.so")
    except Exception:
        hook = None
    mod = types.ModuleType("antenv.axon_hooks")
    state = {"h": hook}
    mod.get_axon_ntff_profile_hook = lambda: state["h"]
    mod.set_axon_ntff_profile_hook = lambda h: state.__setitem__("h", h)
    sys.modules["antenv.axon_hooks"] = mod
    try:
        antenv.axon_hooks = mod
    except Exception:
        pass


def kernel(x_hat, points, likelihoods):
    from concourse.bass_utils import run_bass_kernel_spmd
    global LAST_RESULTS

    trace = bool(int(os.environ.get("CHAMFER_TRACE", "0")))
    if trace:
        _install_axon_profile_hook()

    if "nc" not in _CACHE:
        _CACHE["nc"] = _build_bass()
    nc = _CACHE["nc"]

    in_maps, metas = [], []
    for core in range(8):
        b, d = core // 2, core % 2
        X = x_hat[b] if d == 0 else points[b]
        Y = points[b] if d == 0 else x_hat[b]
        m, meta = _prep_core(np.asarray(X), np.asarray(Y))
        in_maps.append(m)
        metas.append(meta)

    res = run_bass_kernel_spmd(
        nc, in_maps, core_ids=list(range(8)), trace=trace,
    )
    LAST_RESULTS = res

    sums = [_post_core(res.results[c]["out"], metas[c]) for c in range(8)]
    cham_x = sum(sums[c] for c in range(8) if c % 2 == 0) / (B * M)
    cham_y = sum(sums[c] for c in range(8) if c % 2 == 1) / (B * P)
    rec = cham_x + cham_y

    lik = np.asarray(likelihoods, dtype=np.float64)
    bpp = np.log2(lik).sum() / (-(B * P))

    loss = bpp + LMBDA * rec
    return np.array([loss, bpp, rec], dtype=np.float32)


# revision 15
# speedup vs baseline: 1.5146x; 1.1082x over previous
"""Chamfer rate-distortion loss on 8 TRN2 NeuronCores — v3b.

Layout: 8 cores = 4 batches x 2 chamfer directions. Each core computes, for
its (batch, direction), the per-point nearest-neighbor squared distance of
8192 query points X against 8192 reference points Y.

Device algorithm per core:
  - PSUM holds SCALE^2 * |x-y|^2 >= 0 directly: K=13 fp16 hi/lo matmul rows
    (-2ac, -2ae, -2bc cross terms + y^2 hi/lo vs 1 + x^2 hi/lo vs 1).
  - BAND=256 (PAD=112) sorted bands over 64 blocks of 128 queries.
  - Two reduce lanes drain the single-read-ported PSUM in parallel:
      D lane (11 tiles of 4 blocks, 2 PSUM banks): DVE tensor_reduce(min)
        straight from PSUM (~1.19us/tile).
      S lane (10 tiles of 2 blocks, 1 PSUM bank): ScalarE softmin — one
        activation(Exp, scale=-16, accum_out) per block gives
        S_q = sum_j exp(-16384 d_qj); host recovers min ~= -ln(S)/16384 +
        corr, with corr calibrated per-core from two blocks computed by
        BOTH lanes (kills the softmin bias; residual ~2e-5/point).
  - Queries whose NN may fall outside their band (host Morton certificate,
    need > PAD) are gathered into 8 overflow chunks of 32; each chunk scans
    a host-chosen 1024-wide rank window with an exact DVE min.
  - Far-point padding at band edges (never a min; its exp underflows to 0).
  - Head-first DMA with issues split across the Sync and ScalarE queues;
    output DMA in 3 pieces; a dummy Exp prefetches the ACT table at t0.

Soundness: the Morton certificate proves need<=PAD queries have their NN
inside the band; hard queries are covered by their overflow window or
recomputed exactly on host; every query is checked against
est <= d_cap^2 + margin, with host recompute of violators (including all
softmin underflows, which decode to +inf).
"""

import os

import numpy as np

B, M, P = 4, 8192, 8192
SUB = 32
PAD = 112
BAND = SUB + 2 * PAD          # 256
NBLK = 64                     # blocks of 128 queries
NOFCH = 8                     # overflow chunks of 32 hard queries
OFW = 1024                    # overflow window width (2 x 512)
NOFT = NOFCH // 4             # overflow tiles
KROWS = 13
SCALE = 32.0
S2 = SCALE * SCALE            # 1024
ACT_SCALE = -16.0             # exp(-16 * PSUM) = exp(-16384 * d)
SPRIME = -ACT_SCALE * S2      # 16384
LMBDA = 5.0

WT_W = M + NOFCH * 32                 # band stationary | OF stationary
RT_W = P + 2 * PAD                    # far | sorted Y | far
OF_W = NOFCH * OFW
TOT_W = WT_W + RT_W + OF_W
RT0 = WT_W
OF0 = WT_W + RT_W

# tile pattern: 11 'D' tiles (4 blocks, DVE exact) interleaved with 10 'S'
# tiles (2 blocks, ScalarE softmin); blocks are assigned in order, so output
# column b always holds block b (min-PSUM for D, softmin-S for S).
PATTERN = "DSDSDSDSDSDSDSDSDSDSD"
CAL_STILES = (2, 7)           # s-tile ordinals whose block 0 is also DVE'd
OFCOL = 64 + len(CAL_STILES)  # 66
NOUT = 68                     # 64 band | 2 calib | 2 OF

_SOFT_BLOCKS = []
_CAL_BLOCKS = []
_b = 0
_si = 0
for _t in PATTERN:
    if _t == "D":
        _b += 4
    else:
        _SOFT_BLOCKS.extend([_b, _b + 1])
        if _si in CAL_STILES:
            _CAL_BLOCKS.append(_b)
        _si += 1
        _b += 2
SOFT_BLOCKS = frozenset(_SOFT_BLOCKS)
CAL_BLOCKS = tuple(_CAL_BLOCKS)

_CACHE = {}
LAST_RESULTS = None


def _build_bass():
    import concourse.tile as tile
    from concourse import bacc, mybir

    nc = bacc.Bacc(None, target_bir_lowering=False, debug=False)
    f32 = mybir.dt.float32
    f16 = mybir.dt.float16

    wr_d = nc.dram_tensor("wr", [KROWS, TOT_W], f16, kind="ExternalInput")
    out_d = nc.dram_tensor("out", [128, NOUT], f32, kind="ExternalOutput")

    with tile.TileContext(nc) as tc:
        with (
            tc.tile_pool(name="const", bufs=1) as cpool,
            tc.tile_pool(name="outp", bufs=1) as opool,
            tc.tile_pool(name="scr", bufs=2) as spool,
            tc.tile_pool(name="psum_d", bufs=2, space="PSUM") as ppool_d,
            tc.tile_pool(name="psum_s", bufs=4, space="PSUM") as ppool_s,
        ):
            wr = cpool.tile([KROWS, TOT_W], f16)
            # head pieces first (sync queue); bulk via the scalar queue so
            # the issues overlap and the PE can start early
            nc.sync.dma_start(wr[:, 0:512], wr_d[:, 0:512])
            nc.sync.dma_start(wr[:, RT0:RT0 + 768], wr_d[:, RT0:RT0 + 768])
            nc.scalar.dma_start(wr[:, 512:4608], wr_d[:, 512:4608])
            nc.sync.dma_start(wr[:, RT0 + 768:RT0 + 4864],
                              wr_d[:, RT0 + 768:RT0 + 4864])
            nc.scalar.dma_start(wr[:, 4608:RT0], wr_d[:, 4608:RT0])
            nc.sync.dma_start(wr[:, RT0 + 4864:OF0], wr_d[:, RT0 + 4864:OF0])
            nc.scalar.dma_start(wr[:, OF0:], wr_d[:, OF0:])

            outt = opool.tile([128, NOUT], f32)

            # dummy Exp pulls the ACT table load off the critical path
            warm = spool.tile([128, 1], f32, tag="warm")
            nc.vector.memset(warm[:], 0.0)
            nc.scalar.activation(warm[:], warm[:],
                                 mybir.ActivationFunctionType.Exp, scale=1.0)

            blk = 0
            si = 0
            half_sent = False
            for typ in PATTERN:
                if typ == "D":
                    ps = ppool_d.tile([128, 2, 2, BAND], f32, tag="ps")
                    for j in range(4):
                        for s in range(4):
                            u = 4 * (blk + j) + s
                            nc.tensor.matmul(
                                ps[32 * s:32 * s + 32, j // 2, j % 2, :],
                                wr[:, SUB * u:SUB * u + SUB],
                                wr[:, RT0 + SUB * u:RT0 + SUB * u + BAND],
                                start=True, stop=True,
                                tile_position=(0, 32 * s),
                            )
                    nc.vector.tensor_reduce(
                        outt[:, blk:blk + 4], ps[:],
                        axis=mybir.AxisListType.X, op=mybir.AluOpType.min)
                    blk += 4
                else:
                    ps = ppool_s.tile([128, 1, 2, BAND], f32, tag="ps")
                    for j in range(2):
                        for s in range(4):
                            u = 4 * (blk + j) + s
                            nc.tensor.matmul(
                                ps[32 * s:32 * s + 32, 0, j, :],
                                wr[:, SUB * u:SUB * u + SUB],
                                wr[:, RT0 + SUB * u:RT0 + SUB * u + BAND],
                                start=True, stop=True,
                                tile_position=(0, 32 * s),
                            )
                    scr = spool.tile([128, 2, BAND], f32, tag="scr")
                    for j in range(2):
                        nc.scalar.activation(
                            scr[:, j, :], ps[:, 0, j, :],
                            mybir.ActivationFunctionType.Exp,
                            scale=ACT_SCALE,
                            accum_out=outt[:, blk + j:blk + j + 1],
                        )
                    if si in CAL_STILES:
                        cj = CAL_STILES.index(si)
                        nc.vector.tensor_reduce(
                            outt[:, 64 + cj:65 + cj], ps[:, 0, 0, :],
                            axis=mybir.AxisListType.X, op=mybir.AluOpType.min)
                    si += 1
                    blk += 2
                if blk >= 32 and not half_sent:
                    half_sent = True
                    nc.sync.dma_start(out_d[:, 0:32], outt[:, 0:32])

            for t in range(NOFT):
                ps = ppool_d.tile([128, 2, 2, BAND], f32, tag="ps")
                for c in range(4):
                    cc = 4 * t + c
                    for h in range(2):
                        nc.tensor.matmul(
                            ps[32 * c:32 * c + 32, h, :, :],
                            wr[:, M + 32 * cc:M + 32 * cc + 32],
                            wr[:, OF0 + OFW * cc + 512 * h:
                                  OF0 + OFW * cc + 512 * h + 512],
                            start=True, stop=True,
                            tile_position=(0, 32 * c),
                        )
                nc.vector.tensor_reduce(
                    outt[:, OFCOL + t:OFCOL + t + 1], ps[:],
                    axis=mybir.AxisListType.XYZ, op=mybir.AluOpType.min)
                if t == 0:
                    nc.sync.dma_start(out_d[:, 32:64], outt[:, 32:64])

            nc.sync.dma_start(out_d[:, 64:NOUT], outt[:, 64:NOUT])
    nc.compile()
    return nc


def _morton_key(pts):
    rng = pts.max(0) - pts.min(0)
    q = ((pts - pts.min(0)) / (rng + 1e-9) * 1023).astype(np.uint64)

    def spread(x):
        x = x & np.uint64(0x3FF)
        x = (x | (x << np.uint64(16))) & np.uint64(0x30000FF)
        x = (x | (x << np.uint64(8))) & np.uint64(0x300F00F)
        x = (x | (x << np.uint64(4))) & np.uint64(0x30C30C3)
        x = (x | (x << np.uint64(2))) & np.uint64(0x9249249)
        return x

    return (spread(q[:, 0]) | (spread(q[:, 1]) << np.uint64(1))
            | (spread(q[:, 2]) << np.uint64(2)))


def _prep_core(X, Y):
    """Host prep for one (batch, direction)."""
    X64 = X.astype(np.float64)
    Y64 = Y.astype(np.float64)

    # Morton d_cap (axis-independent NN upper bound from 32 candidates)
    allpts = np.concatenate([X64, Y64])
    mk = _morton_key(allpts)
    inv = np.empty(2 * M, dtype=np.int64)
    inv[np.argsort(mk, kind="stable")] = np.arange(2 * M)
    y_rank = inv[M:]
    order_y = np.argsort(y_rank, kind="stable")
    sorted_ranks = y_rank[order_y]
    idx = np.searchsorted(sorted_ranks, inv[:M])
    cand = np.clip(idx[:, None] + np.arange(-16, 16)[None, :], 0, M - 1)
    cands = order_y[cand]
    d_cap2 = ((X64[:, None, :] - Y64[cands]) ** 2).sum(-1).min(1)
    d_cap = np.sqrt(d_cap2 / 0.98)

    # choose the sort axis with the fewest hard queries
    i = np.arange(M)
    best = None
    for axis in range(3):
        xo = np.argsort(X[:, axis], kind="stable")
        yo = np.argsort(Y[:, axis], kind="stable")
        zx = X64[xo, axis]
        zy = Y64[yo, axis]
        dc = d_cap[xo]
        lo_idx = np.searchsorted(zy, zx - dc)
        hi_idx = np.searchsorted(zy, zx + dc)
        cch = i // SUB
        need = np.maximum(np.maximum(SUB * cch - lo_idx,
                                     hi_idx - (SUB * cch + SUB)), 0)
        nhard = int((need > PAD).sum())
        if best is None or nhard < best[0]:
            best = (nhard, axis, xo, yo, lo_idx, hi_idx, need)
    _, axis, xo, yo, lo_idx, hi_idx, need = best

    Xs = X64[xo]
    Ys = Y64[yo]
    X2 = (Xs ** 2).sum(1)
    Y2 = (Ys ** 2).sum(1)

    # far pad point: beyond data range along the sort axis, never a min
    zfar = np.abs(np.concatenate([Xs[:, axis], Ys[:, axis]])).max() + 2.0
    ypad = np.zeros(3)
    ypad[axis] = zfar
    Yx = np.vstack([Ys, ypad])    # index P = pad

    # fp16 hi/lo decomposition
    Xss = SCALE * Xs
    Yss = SCALE * Yx
    a = Xss.astype(np.float16)
    bb = (Xss - a.astype(np.float64)).astype(np.float16)
    c = Yss.astype(np.float16)
    e = (Yss - c.astype(np.float64)).astype(np.float16)
    w = (Yss ** 2).sum(1)
    wh = w.astype(np.float16)
    wl = (w - wh.astype(np.float64)).astype(np.float16)
    v = (Xss ** 2).sum(1)
    vh = v.astype(np.float16)
    vl = (v - vh.astype(np.float64)).astype(np.float16)
    na = (-2.0 * a.astype(np.float64)).astype(np.float16)
    nb = (-2.0 * bb.astype(np.float64)).astype(np.float16)

    # hard queries -> overflow chunks with per-chunk 1024 windows
    hard = np.flatnonzero(need > PAD)
    of_idx = np.zeros(NOFCH * 32, dtype=np.int64)     # query per slot
    of_valid = np.zeros(NOFCH * 32, dtype=bool)
    w0s = np.zeros(NOFCH, dtype=np.int64)
    spill = list(hard[NOFCH * 32:])
    for ccn in range(NOFCH):
        grp = hard[32 * ccn:32 * ccn + 32]
        if len(grp) == 0:
            continue
        loi, hii = lo_idx[grp], hi_idx[grp]
        # pick w0 among candidate starts maximizing covered members
        cands_w0 = np.clip(np.concatenate([loi, hii - OFW]), 0, P - OFW)
        covn = ((loi[None, :] >= cands_w0[:, None])
                & (hii[None, :] <= cands_w0[:, None] + OFW)).sum(1)
        w0 = int(cands_w0[covn.argmax()])
        w0s[ccn] = w0
        cov = (loi >= w0) & (hii <= w0 + OFW)
        nslot = len(grp)
        of_idx[32 * ccn:32 * ccn + nslot] = grp
        of_valid[32 * ccn:32 * ccn + nslot] = cov
        spill.extend(grp[~cov])
        if nslot < 32:
            of_idx[32 * ccn + nslot:32 * ccn + 32] = grp[0]

    # device input
    wr = np.zeros((KROWS, TOT_W), dtype=np.float16)
    wt = wr[:, :WT_W]
    rt = wr[:, RT0:RT0 + RT_W]
    ofr = wr[:, OF0:]

    wt[0:3, :M] = na.T
    wt[3:6, :M] = na.T
    wt[6:9, :M] = nb.T
    wt[9:11, :M] = 1.0
    wt[11, :M] = vh
    wt[12, :M] = vl
    wt[0:3, M:] = na[of_idx].T
    wt[3:6, M:] = na[of_idx].T
    wt[6:9, M:] = nb[of_idx].T
    wt[9:11, M:] = 1.0
    wt[11, M:] = vh[of_idx]
    wt[12, M:] = vl[of_idx]

    # rt: [0:PAD]=far pad, [PAD:PAD+P]=sorted Y, [PAD+P:]=far pad
    ridx = np.full(RT_W, P, dtype=np.int64)
    ridx[PAD:PAD + P] = np.arange(P)
    rt[0:3, :] = c[ridx].T
    rt[3:6, :] = e[ridx].T
    rt[6:9, :] = c[ridx].T
    rt[9, :] = wh[ridx]
    rt[10, :] = wl[ridx]
    rt[11:13, :] = 1.0

    # overflow windows (real Y columns, no pad needed)
    oidx = (w0s[:, None] + np.arange(OFW)[None, :]).reshape(-1)
    ofr[0:3, :] = c[oidx].T
    ofr[3:6, :] = e[oidx].T
    ofr[6:9, :] = c[oidx].T
    ofr[9, :] = wh[oidx]
    ofr[10, :] = wl[oidx]
    ofr[11:13, :] = 1.0

    return {"wr": wr}, {
        "Xs": Xs, "Ys": Ys, "X2": X2, "Y2": Y2,
        "hard": hard, "of_idx": of_idx, "of_valid": of_valid,
        "d_cap2": d_cap2[xo],
        "spill": np.array(sorted(set(int(s) for s in spill)), dtype=np.int64),
    }


def _post_core(out, meta):
    """Combine device output into sum over queries of min-D (float64)."""
    out = out.astype(np.float64)
    est = np.empty(M)
    softq = np.zeros(M, dtype=bool)
    for b in range(NBLK):
        vals = out[:, b]
        sl = slice(128 * b, 128 * b + 128)
        if b in SOFT_BLOCKS:
            with np.errstate(divide="ignore"):
                est[sl] = np.where(vals > 0.0,
                                   -np.log(np.maximum(vals, 1e-300)) / SPRIME,
                                   np.inf)
            softq[sl] = True
        else:
            est[sl] = vals / S2

    # softmin bias calibration from the doubly-computed blocks
    diffs = []
    for cj, b in enumerate(CAL_BLOCKS):
        exact = out[:, 64 + cj] / S2
        soft = est[128 * b:128 * b + 128]
        ok = np.isfinite(soft)
        diffs.append((exact - soft)[ok])
    dall = np.concatenate(diffs)
    corr = dall.mean() if len(dall) else 0.0
    est[softq] += corr

    # overflow results (exact min over each chunk window)
    for t in range(NOFT):
        vals = out[:, OFCOL + t] / S2
        for p in range(128):
            slot = 128 * t + p
            if not meta["of_valid"][slot]:
                continue
            q = meta["of_idx"][slot]
            if vals[p] < est[q]:
                est[q] = vals[p]

    # host-exact fixes: spill + softmin underflows + d_cap sanity violations
    fix = set(int(q) for q in meta["spill"])
    fix.update(int(q) for q in
               np.flatnonzero(~(est <= meta["d_cap2"] / 0.98 + 2e-4)))
    if fix:
        qq = np.array(sorted(fix), dtype=np.int64)
        D = (meta["Y2"][None, :] - 2.0 * (meta["Xs"][qq] @ meta["Ys"].T))
        est[qq] = D.min(1) + meta["X2"][qq]
    return est.sum()


def _install_axon_profile_hook():
    import sys
    import types
    try:
        from antenv.axon_hooks import get_axon_ntff_profile_hook  # noqa: F401
        return
    except ImportError:
        pass
    try:
        import antenv
        from trn_agent_boot.trn_boot import _ntff_profile_via_ctypes
        hook = _ntff_profile_via_ctypes("/opt/axon/libaxon_pjrt.so")
    except Exception:
        hook = None
    mod = types.ModuleType("antenv.axon_hooks")
    state = {"h": hook}
    mod.get_axon_ntff_profile_hook = lambda: state["h"]
    mod.set_axon_ntff_profile_hook = lambda h: state.__setitem__("h", h)
    sys.modules["antenv.axon_hooks"] = mod
    try:
        antenv.axon_hooks = mod
    except Exception:
        pass


def kernel(x_hat, points, likelihoods):
    from concourse.bass_utils import run_bass_kernel_spmd
    global LAST_RESULTS

    trace = bool(int(os.environ.get("CHAMFER_TRACE", "0")))
    if trace:
        _install_axon_profile_hook()

    if "nc" not in _CACHE:
        _CACHE["nc"] = _build_bass()
    nc = _CACHE["nc"]

    in_maps, metas = [], []
    for core in range(8):
        b, d = core // 2, core % 2
        X = x_hat[b] if d == 0 else points[b]
        Y = points[b] if d == 0 else x_hat[b]
        m, meta = _prep_core(np.asarray(X), np.asarray(Y))
        in_maps.append(m)
        metas.append(meta)

    res = run_bass_kernel_spmd(
        nc, in_maps, core_ids=list(range(8)), trace=trace,
    )
    LAST_RESULTS = res

    sums = [_post_core(res.results[c]["out"], metas[c]) for c in range(8)]
    cham_x = sum(sums[c] for c in range(8) if c % 2 == 0) / (B * M)
    cham_y = sum(sums[c] for c in range(8) if c % 2 == 1) / (B * P)
    rec = cham_x + cham_y

    lik = np.asarray(likelihoods, dtype=np.float64)
    bpp = np.log2(lik).sum() / (-(B * P))

    loss = bpp + LMBDA * rec
    return np.array([loss, bpp, rec], dtype=np.float32)
